# revision 66
# baseline (speedup 1.0000x reference)
"""Multi-head attention (B=4, G=2048, E=768, H=4) on 8 TRN2 NeuronCores.

Sharding: core c = (batch b = c//2, head-pair hp = c%2). Each core computes
Q/K/V for only its 2 heads over all 2048 tokens (this removes the K/V
duplication a query-split sharding would have), attention for those heads
over all 2048 queries, and a PARTIAL output projection (contraction over
its 384 av-rows, bf16). The pairwise sum of the two head-pair partials
happens HOST-side during unshard (fp32) -- an on-chip ReduceScatter was
measured at ~17 us per 1.5 MB chunk plus end-of-kernel serialization,
costing more than the entire PE-time win.

All heavy matmuls are fp16 with fp32 PSUM accumulation. fp8 DoubleRow was
evaluated and rejected: on hardware a DR matmul streams moving columns at
the same 1 col/cycle as fp16 (it only doubles contraction per instruction),
and these matmuls are column-bound, so fp8 + residual compensation loses.

Biases: b_qkv is zero by the input spec; with zero Q-bias the K-bias is
softmax-invariant, and the V-bias folds exactly into b_proj
(bp_eff = bp/2 + bv_loc @ Wp_loc), so no on-chip qkv bias adds are needed.

Device dataflow:
  xT resident in SBUF (contraction dim on partitions), natural token order.
  V phase:   Vext[h][tt] (128 tok, 192+ones) = x @ Wv_loc per head/tile.
  QK phase:  kt/qt tiles (128 c, 2048 tok) = (x @ W)^T, 3 c-blocks each.
  Attention: per (512-query block qb, local head h): ET = K Q^T by 128-key
             tiles -> exp on ACT (scale 1/sqrt(768)) -> avT = V^T @ att;
             row sums ride the ones column; normalization is emitted one
             (qb,h)-unit late so the PE never waits on the DVE chain.
  Proj:      partial y(q,e) = avs^T @ Wp_loc + bp_eff -> y (bf16) per
             128-query tile.
"""
import sys

sys.path.insert(0, "/opt/trn_rl_repo")
sys.path.insert(0, "/root/.axon_site")

from contextlib import ExitStack

import numpy as np

import concourse.bass as bass
import concourse.tile as tile
from concourse import bacc, mybir
from concourse.bass_utils import run_bass_kernel_spmd

N_CORES = 8
B, G, E, H = 4, 2048, 768, 4
D = E // H            # 192
HL = 2                # local heads per core
CL = HL * D           # 384 local c-rows
HALF = G // 2         # 1024 queries owned per core after scatter
KCH = E // 128        # 6 contraction chunks
SCALE = 1.0 / float(np.sqrt(E))
PAIRS = [[0, 1], [2, 3], [4, 5], [6, 7]]

f32 = mybir.dt.float32
f16 = mybir.dt.float16
bf16 = mybir.dt.bfloat16
f8 = mybir.dt.float8e4
DR = mybir.MatmulPerfMode.DoubleRow


def _emit(nc, t):
    with ExitStack() as top:
        tc = top.enter_context(tile.TileContext(nc))
        const = top.enter_context(tc.tile_pool(name="const", bufs=1))
        kqt_p = top.enter_context(tc.tile_pool(name="kqt", bufs=1))
        v_p = top.enter_context(tc.tile_pool(name="vext", bufs=1))
        dram = top.enter_context(tc.tile_pool(name="dram", bufs=1, space="DRAM"))

        ones1 = const.tile([1, 128], f32, tag="ones1")
        nc.vector.memset(ones1[:], 1.0)
        warmsrc = const.tile([1, 512], f16, tag="warmsrc")
        nc.vector.memset(warmsrc[:], 0.0)
        ones1h = const.tile([1, 128], f16, tag="ones1h")
        nc.vector.memset(ones1h[:], 1.0)
        bp_sb = const.tile([1, E], f32, tag="bp")
        nc.gpsimd.dma_start(bp_sb[:], t["bp"][:])
        bp16 = const.tile([1, E], f16, tag="bp16")
        nc.vector.tensor_copy(bp16[:], bp_sb[:])
        bp_bc = const.tile([128, E], f32, tag="bp_bc")
        wqk_sb = [kqt_p.tile([128, KCH * 128], f16, tag=f"wqk{i}", name=f"wqk{i}")
                  for i in range(6)]

        kt_sb = [kqt_p.tile([128, G], f16, tag=f"kt{i}", name=f"kt{i}")
                 for i in range(3)]
        qt_sb = [kqt_p.tile([128, G], f16, tag=f"qt{i}", name=f"qt{i}")
                 for i in range(3)]

        # fp16 V per (head, token-tile): [tok-part, 192 + ones column]
        vext = [[v_p.tile([128, D + 1], f16, tag=f"v{h}_{tt}", name=f"v{h}_{tt}")
                 for tt in range(16)] for h in range(HL)]
        for h in range(HL):
            for tt in range(16):
                nc.vector.memset(vext[h][tt][:, D:D + 1], 1.0)



        with tc.tile_pool(name="xt_pool", bufs=1) as xt_p:
            xt = xt_p.tile([128, KCH * G], f16, tag="xt")

            # ---- V phase --------------------------------------------------
            with tc.tile_pool(name="vps", bufs=4, space="PSUM") as vps, \
                 tc.tile_pool(name="wv_pool", bufs=1) as wvp:
                wv_sb = wvp.tile([128, KCH * CL], f16, tag="wv")
                # interleave wv/xt chunk loads so chunk k of both arrives
                # early enough for the k-th accumulation burst; first chunk
                # split so the V phase can start on a quarter of it; wqk
                # tiles ride between later chunks so QK never waits on them
                for k in range(KCH):
                    nc.sync.dma_start(wv_sb[:, k * CL:(k + 1) * CL],
                                      t["wv"][:, k * CL:(k + 1) * CL])
                    if k == 0:
                        for q in range(4):
                            nc.sync.dma_start(
                                xt[:, q * 512:(q + 1) * 512],
                                t["xt"][:, q * 512:(q + 1) * 512])
                    else:
                        nc.sync.dma_start(xt[:, k * G:(k + 1) * G],
                                          t["xt"][:, k * G:(k + 1) * G])
                    for tb in {2: [0], 3: [1], 4: [2], 5: [3, 4, 5]}.get(k, []):
                        nc.sync.dma_start(wqk_sb[tb][:],
                                          t["wqk"][:, tb * 768:(tb + 1) * 768])

                # PE warm-up during the initial DMA wait: dependency-free
                # fp16 matmuls keep the tensor engine busy so the DVFS ramp
                # to full clock completes before the real work arrives
                # (without it the whole V phase runs at the 1.2GHz p-state)
                warm = vps.tile([128, 512], f32, tag="vb", name="warm")
                for _ in range(16):
                    nc.tensor.matmul(warm[:], ones1h[:], warmsrc[:],
                                     start=True, stop=True)

                # token-tile groups of 4, k outer within the group: the k-th
                # burst only needs DMA chunk k -> PE tracks DMA arrival
                for tg in range(4):
                    pas = [vps.tile([128, CL], f32, tag="va", name=f"pa{i}")
                           for i in range(4)]
                    for k in range(KCH):
                        for i in range(4):
                            tt = tg * 4 + i
                            lhsT = xt[:, k * G + tt * 128: k * G + tt * 128 + 128]
                            nc.tensor.matmul(pas[i][:], lhsT,
                                             wv_sb[:, k * CL: k * CL + CL],
                                             start=(k == 0), stop=(k == KCH - 1))
                    if tg == 0:
                        # fp16 broadcast: an fp32 matmul is 4 cycles/row and
                        # this runs inside the slow pre-ramp clock window
                        for j in range(2):
                            bb = vps.tile([128, 384], f32, tag="vb")
                            nc.tensor.matmul(bb[:], ones1h[:],
                                             bp16[:, j * 384:(j + 1) * 384],
                                             start=True, stop=True)
                            nc.vector.tensor_copy(bp_bc[:, j * 384:(j + 1) * 384],
                                                  bb[:])
                    for i in range(4):
                        tt = tg * 4 + i
                        for h in range(HL):
                            nc.vector.tensor_copy(vext[h][tt][:, 0:D],
                                                  pas[i][:, h * D: h * D + D])

            # ---- QK phase -------------------------------------------------
            with tc.tile_pool(name="qkps", bufs=3, space="PSUM") as qkps:
                for tblk in range(6):
                    wt = wqk_sb[tblk]
                    dest = kt_sb[tblk] if tblk < 3 else qt_sb[tblk - 3]
                    for n in range(4):
                        ps = qkps.tile([128, 512], f32, tag="qk")
                        tok0 = n * 512
                        for k in range(KCH):
                            nc.tensor.matmul(
                                ps[:], wt[:, k * 128:(k + 1) * 128],
                                xt[:, k * G + tok0: k * G + tok0 + 512],
                                start=(k == 0), stop=(k == KCH - 1))
                        nc.vector.tensor_copy(dest[:, tok0:tok0 + 512],
                                              ps[:, 0:512])

        # ---- attention + partial projection (xt freed) -------------------
        with tc.tile_pool(name="etps", bufs=3, space="PSUM") as et_ps, \
             tc.tile_pool(name="bcps", bufs=1, space="PSUM") as bc_ps, \
             tc.tile_pool(name="avps", bufs=2, space="PSUM") as av_ps, \
             tc.tile_pool(name="att_pool", bufs=3) as att_p, \
             tc.tile_pool(name="att8_pool", bufs=2) as att8_p, \
             tc.tile_pool(name="avs_pool", bufs=2) as avs_p, \
             tc.tile_pool(name="r_pool", bufs=2) as r_p, \
             tc.tile_pool(name="r1_pool", bufs=1) as r1_p, \
             tc.tile_pool(name="out_pool", bufs=4) as out_p, \
             tc.tile_pool(name="wp_pool", bufs=1) as wpp:
            wp_sb = wpp.tile([128, 4 * E], f16, tag="wp")
            nc.sync.dma_start(wp_sb[:], t["wp"][:])

            avs_tiles = {}

            def _c_chunks(h):
                out = []
                c, c1 = h * D, (h + 1) * D
                while c < c1:
                    ti, off = divmod(c, 128)
                    ln = min(128 - off, c1 - c)
                    out.append((ti, off, ln))
                    c += ln
                return out

            def attn_head(qb, h):
                avT0 = av_ps.tile([128, 512], f32, tag="avT0", name="avT0")
                avT1 = av_ps.tile([65, 512], f32, tag="avT1", name="avT1")
                chunks = _c_chunks(h)
                for kc in range(16):
                    et = et_ps.tile([128, 512], f32, tag="et", name="et")
                    for ci, (ti, off, ln) in enumerate(chunks):
                        nc.tensor.matmul(
                            et[:],
                            kt_sb[ti][off:off + ln, kc * 128:(kc + 1) * 128],
                            qt_sb[ti][off:off + ln, qb * 512:(qb + 1) * 512],
                            start=(ci == 0), stop=(ci == len(chunks) - 1))
                    att = att_p.tile([128, 512], f16, tag="att", name="att")
                    nc.scalar.activation(att[:], et[:],
                                         mybir.ActivationFunctionType.Exp,
                                         scale=SCALE)
                    vt = vext[h][kc]
                    nc.tensor.matmul(avT0[:], vt[:, 0:128], att[:],
                                     start=(kc == 0), stop=(kc == 15))
                    nc.tensor.matmul(avT1[:], vt[:, 128:193], att[:],
                                     start=(kc == 0), stop=(kc == 15))
                # start the reciprocal chain now (DVE+DMA only, no PE); the
                # bounce DMA rides the idle gpsimd queue so it never sits
                # behind osb output DMAs on the sync queue
                s64 = r1_p.tile([65, 512], f32, tag="s64", name="s64")
                nc.vector.tensor_copy(s64[64:65, :], avT1[64:65, :])
                r0 = r1_p.tile([1, 512], f32, tag="r0", name="r0")
                nc.gpsimd.dma_start(r0[:], s64[64:65, :])
                rr = r_p.tile([1, 512], f32, tag="rr", name="rr")
                nc.vector.reciprocal_approx_fast(rr[:], r0[:])
                return avT0, avT1, rr

            def normalize(qb, h, avT0, avT1, rr):
                # PE broadcast of 1/sums across partitions, in fp16 (an fp32
                # matmul runs at 4 cycles/row) and in its own PSUM bank so it
                # never waits on the proj tiles' output-DMA recycling
                rr16 = r_p.tile([1, 512], f16, tag="rr16", name="rr16")
                nc.vector.tensor_copy(rr16[:], rr[:])
                bc = bc_ps.tile([128, 512], f32, tag="bc", name="bc")
                nc.tensor.matmul(bc[:], ones1h[:], rr16[:], start=True, stop=True)
                bc_sb = r1_p.tile([128, 512], f32, tag="bcsb", name="bcsb")
                nc.vector.tensor_copy(bc_sb[:], bc[:])
                for dc, (avT, rows) in enumerate(((avT0, 128), (avT1, 64))):
                    avs = avs_p.tile([rows, 512], f16, tag=f"avs{h}_{dc}",
                                     name=f"avs{h}_{dc}")
                    nc.vector.tensor_mul(avs[:], avT[0:rows, :], bc_sb[0:rows, :])
                    avs_tiles[(qb, h * 2 + dc)] = avs

            def proj(qb, pre_hook=None):
                # partial projection over the local 384 c-rows; the pairwise
                # sum over head-pairs happens host-side during unshard
                for qs in range(4):
                    p0 = et_ps.tile([128, 384], f32, tag="et", name="p0")
                    p1 = et_ps.tile([128, 384], f32, tag="et", name="p1")
                    for cc in range(4):
                        if pre_hook is not None and qs == 0 and cc == 2:
                            pre_hook()
                        rows = 128 if cc % 2 == 0 else 64
                        lhsT = avs_tiles[(qb, cc)][:, qs * 128:(qs + 1) * 128]
                        nc.tensor.matmul(p0[:], lhsT,
                                         wp_sb[0:rows, cc * 768: cc * 768 + 384],
                                         start=(cc == 0), stop=(cc == 3))
                        nc.tensor.matmul(p1[:], lhsT,
                                         wp_sb[0:rows, cc * 768 + 384: cc * 768 + 768],
                                         start=(cc == 0), stop=(cc == 3))
                    osb = out_p.tile([128, E], bf16, tag="osb", name="osb")
                    nc.vector.tensor_add(osb[:, 0:384], p0[:], bp_bc[:, 0:384])
                    nc.vector.tensor_add(osb[:, 384:768], p1[:], bp_bc[:, 384:768])
                    row = qb * 512 + qs * 128
                    nc.sync.dma_start(t["y"][row:row + 128, :], osb[:])

            pending = None
            for qb in range(4):
                for h in range(HL):
                    result = attn_head(qb, h)
                    if pending is not None:
                        pqb, ph, pavT0, pavT1, prr = pending
                        normalize(pqb, ph, pavT0, pavT1, prr)
                        if ph == HL - 1:
                            proj(pqb)
                    pending = (qb, h) + result
            pqb, ph, pavT0, pavT1, prr = pending
            proj(pqb, pre_hook=lambda: normalize(pqb, ph, pavT0, pavT1, prr))


_CACHED_NC = None


def _get_nc():
    global _CACHED_NC
    if _CACHED_NC is None:
        nc = bacc.Bacc("TRN2", target_bir_lowering=False, debug=False,
                       num_devices=N_CORES)
        t = {
            "xt": nc.dram_tensor("xt", (128, KCH * G), f16, kind="ExternalInput").ap(),
            "wqk": nc.dram_tensor("wqk", (128, 6 * 768), f16, kind="ExternalInput").ap(),
            "wv": nc.dram_tensor("wv", (128, KCH * CL), f16, kind="ExternalInput").ap(),
            "wp": nc.dram_tensor("wp", (128, 4 * E), f16, kind="ExternalInput").ap(),
            "bp": nc.dram_tensor("bp", (1, E), f32, kind="ExternalInput").ap(),
            "y": nc.dram_tensor("y", (G, E), bf16, kind="ExternalOutput").ap(),
        }
        _emit(nc, t)
        nc.compile()
        _CACHED_NC = nc
    return _CACHED_NC


def _pack_contraction(w, rows=128):
    """(R, C) -> (rows, R//rows * C): contraction chunks on partitions,
    per-partition data contiguous (k-major along free dim)."""
    r, c = w.shape
    n = r // rows
    return np.ascontiguousarray(
        w.reshape(n, rows, c).transpose(1, 0, 2).reshape(rows, n * c))


def make_in_maps(x, W_qkv, b_qkv, W_proj, b_proj):
    x = np.asarray(x, dtype=np.float32)
    W_qkv = np.asarray(W_qkv, dtype=np.float32)
    b_qkv = np.asarray(b_qkv, dtype=np.float32)
    W_proj = np.asarray(W_proj, dtype=np.float32)
    b_proj = np.asarray(b_proj, dtype=np.float32)

    # qkv column factorization: col = (h, d, {q,k,v}) with qkv fastest.
    # b_qkv is zero by the input spec: with zero Q-bias the K-bias is
    # softmax-invariant, and the V-bias is folded into bp below.
    Wf = W_qkv.reshape(E, H, D, 3)
    bf = b_qkv.reshape(H, D, 3)

    per_hp = []
    for hp in range(2):
        hs = slice(2 * hp, 2 * hp + 2)
        Wq = np.ascontiguousarray(Wf[:, hs, :, 0].reshape(E, CL))
        Wk = np.ascontiguousarray(Wf[:, hs, :, 1].reshape(E, CL))
        Wv = np.ascontiguousarray(Wf[:, hs, :, 2].reshape(E, CL))
        bv = np.ascontiguousarray(bf[hs, :, 2].reshape(CL))

        blocks = [_pack_contraction(Wk[:, i * 128:(i + 1) * 128]) for i in range(3)]
        blocks += [_pack_contraction(Wq[:, i * 128:(i + 1) * 128]) for i in range(3)]
        wqk = np.concatenate(blocks, axis=1)  # (128, 6*768)

        wv_packed = _pack_contraction(Wv)  # (128, 6*384)
        # local W_proj rows c=(h,d) split per head into 128- and 64-row chunks
        Wp_loc = W_proj[2 * hp * D: (2 * hp + 2) * D]  # (384, 768)
        wp = np.zeros((128, 4, E), dtype=np.float32)
        for h in range(2):
            r0 = h * D
            wp[0:128, 2 * h] = Wp_loc[r0: r0 + 128]
            wp[0:64, 2 * h + 1] = Wp_loc[r0 + 128: r0 + D]
        wp = np.ascontiguousarray(wp.reshape(128, 4 * E))
        # halve b_proj (both partials add it, RS sums the pair) and fold in
        # the local V-bias exactly: av+bv projects to y + bv @ Wp_loc
        bp_eff = 0.5 * b_proj + bv @ Wp_loc
        per_hp.append({
            "wqk": wqk.astype(np.float16), "wv": wv_packed.astype(np.float16),
            "wp": wp.astype(np.float16),
            "bp": bp_eff.reshape(1, E).astype(np.float32),
        })

    in_maps = []
    for c in range(N_CORES):
        b, hp = divmod(c, 2)
        xt = _pack_contraction(np.ascontiguousarray(x[b].T))  # (128, 6*2048)
        in_maps.append({"xt": xt.astype(np.float16), **per_hp[hp]})
    return in_maps


def kernel(**inputs):
    nc = _get_nc()
    in_maps = make_in_maps(inputs["x"], inputs["W_qkv"], inputs["b_qkv"],
                           inputs["W_proj"], inputs["b_proj"])
    res = run_bass_kernel_spmd(nc, in_maps, core_ids=list(range(N_CORES)))
    out = np.empty((B, G, E), dtype=np.float32)
    for b in range(B):
        # unshard: sum the two head-pair partials of each batch
        out[b] = (res.results[2 * b]["y"].astype(np.float32)
                  + res.results[2 * b + 1]["y"].astype(np.float32))
    return out
